# revision 28
# baseline (speedup 1.0000x reference)
"""DividedAttention (TimeSformer-style divided space-time attention) on 8 trn2 cores.

Sharding: pure data-parallel over batch B=16 -> 2 batch items per core.

v3: startup/tail restructure + kT key-padding, from v2's two-batch pipeline.
  - consolidated input DMAs: one dma_start per xT token-chunk (was 4),
    wqkv split so hp0's q/k slices land first; loads spread over the
    scalar+sync queues so issue slots (~850ns each) don't serialize.
  - b0's hp0 attention steps are interleaved INTO the projection stage, so
    the PE has DMA-independent work while the input stream catches up
    (kills the 10us of startup gaps + early p-state resets).
  - vfl->va/vb rearrangement DMAs moved to the gpsimd queue (was sync),
    freeing sync for input loads + output stores.
  - kT padded to 128 key columns per chunk: the S^T chunk-1 matmul pair
    becomes (64,128)-shaped instead of (64,69); the 65..127-partition
    output shape broke the PE's paired half-array mode (211ns vs 110ns
    per pair in the v2 trace).
  - b1 frame order [1..7,0]: only out-tile t=1 and the cls tile t=0
    depend on the last attention step, shrinking the serial tail.
  - out stores in bf16 (halves output DMA; host converts to f32).
"""
import sys

sys.path.insert(0, "/opt/trn_rl_repo")

import numpy as np
import ml_dtypes

from concourse import bacc
import concourse.mybir as mybir
import concourse.tile as tile
from concourse import bass_utils

BF16 = mybir.dt.bfloat16
F32 = mybir.dt.float32
NPBF = ml_dtypes.bfloat16

B, SP, F, DIM, H, DH = 16, 196, 8, 512, 8, 64
INNER = H * DH            # 512
N = 1 + F * SP            # 1569
SP1 = SP + 1              # 197
KPAD = 256                # kT padded key columns (2 chunks of 128)
NCORES = 8
NB = B // NCORES          # 2
KC = DIM // 128           # 4
NT = (N + 127) // 128     # 13
LAST = N - 128 * (NT - 1)  # 33
TCH = [(0, 1 + 2 * SP), (1 + 2 * SP, 2 * SP), (1 + 4 * SP, 2 * SP), (1 + 6 * SP, 2 * SP)]

EXP = mybir.ActivationFunctionType.Exp
ADD = mybir.AluOpType.add
MULT = mybir.AluOpType.mult

LAG = 2
EARING = 4
FORDER1 = [1, 2, 3, 4, 5, 6, 7, 0]   # b1 frame order; f0 last -> tail is t1+t0 only
G1 = 32                              # loop iteration where b1 steps start
NSTEP_L = 32 + 32                    # b0 + b1


def _v_pieces(tok0, length):
    out = []
    done = 0
    while done < length:
        tok = tok0 + done
        t, p0 = divmod(tok, 128)
        l = min(128 - p0, length - done)
        out.append((t, p0, done, l))
        done += l
    return out


def _rearr_by_tile():
    """v_flat tile -> list of (dst_kind, frame, src_p0, dst_p0, len)."""
    by_tile = {}
    for f in range(F):
        for (t, p0, d0, l) in _v_pieces(1 + SP * f, 127):
            by_tile.setdefault(t, []).append(("a", f, p0, 1 + d0, l))
        for (t, p0, d0, l) in _v_pieces(128 + SP * f, 69):
            by_tile.setdefault(t, []).append(("b", f, p0, d0, l))
    return by_tile


def build_nc():
    nc = bacc.Bacc(num_devices=NCORES)

    xT = nc.declare_dram_parameter("xT", [NB, DIM, N], BF16, isOutput=False)
    wqkv = nc.declare_dram_parameter("wqkv", [DIM, 3 * INNER], BF16, isOutput=False)
    wout = nc.declare_dram_parameter("wout", [INNER, DIM], BF16, isOutput=False)
    bout = nc.declare_dram_parameter("bout", [1, DIM], F32, isOutput=False)
    out = nc.declare_dram_parameter("out", [NB, N, DIM], BF16, isOutput=True)

    rearr = _rearr_by_tile()

    with tile.TileContext(nc) as tc:
        with (
            tc.tile_pool(name="const", bufs=1) as const,
            tc.tile_pool(name="perb", bufs=2) as perb,
            tc.tile_pool(name="vflat", bufs=1) as vflat_pool,
            tc.tile_pool(name="clsp", bufs=2) as clsp,
            tc.tile_pool(name="small", bufs=3) as small,
            tc.tile_pool(name="outp", bufs=3) as outp,
            tc.tile_pool(name="ps_proj", bufs=2, space="PSUM") as ps_proj,
            tc.tile_pool(name="ps_st", bufs=2, space="PSUM") as ps_st,
            tc.tile_pool(name="ps_po", bufs=2, space="PSUM") as ps_po,
        ):
            # ---- constants (memory only; DMAs are emitted in the schedule)
            wqkv_sb = const.tile([128, KC, 3 * INNER], BF16)
            wout_sb = const.tile([128, KC, DIM], BF16)
            ones128 = const.tile([128, 64], BF16)
            nc.vector.memset(ones128, 1.0)
            ones_row = const.tile([1, 128], BF16)
            nc.vector.memset(ones_row, 1.0)
            ea_ring = const.tile([128, 2, EARING, 2, SP1], BF16)
            bout_bc = const.tile([128, DIM], F32)
            bout_bf = const.tile([1, DIM], BF16)
            nc.vector.memset(bout_bf, 0.0)

            def load_wqkv_cols(eng, c0, c1):
                eng.dma_start(
                    out=wqkv_sb[:, :, c0:c1],
                    in_=wqkv[:, c0:c1].rearrange("(c p) o -> p c o", p=128))

            S = [dict() for _ in range(NB)]
            _pc = [0]

            def alloc_batch(b):
                st = S[b]
                st["xT"] = perb.tile([128, KC, N], BF16, tag="xt", name=f"xt{b}")
                st["qT"] = perb.tile([128, 4, F, SP1], BF16, tag="qT", name=f"qT{b}")
                st["kT"] = perb.tile([128, 4, F, KPAD], BF16, tag="kT", name=f"kT{b}")
                st["vfl"] = vflat_pool.tile([128, NT, INNER], BF16, tag="vfl", name=f"vfl{b}")
                st["va"] = perb.tile([128, F, INNER], BF16, tag="vfra", name=f"vfra{b}")
                st["vb"] = perb.tile([128, F, INNER], BF16, tag="vfrb", name=f"vfrb{b}")
                st["attnT"] = perb.tile([128, KC, N], BF16, tag="attnT", name=f"attnT{b}")
                oscls = clsp.tile([128, 2, 4], F32, tag="oscls", name=f"oscls{b}")
                st["oscls"] = oscls
                st["ocls"] = oscls[:, 0, :]
                st["scls"] = oscls[:, 1, :]
                st["ecc_row"] = clsp.tile([1, H], BF16, tag="eccrow", name=f"eccrow{b}")
                st["vTcls"] = clsp.tile([128, 4], F32, tag="vTcls", name=f"vTcls{b}")
                st["ecc_bc"] = clsp.tile([128, 4], F32, tag="eccbc", name=f"eccbc{b}")
                st["rcls"] = clsp.tile([128, 4], F32, tag="rcls", name=f"rcls{b}")
                st["tevc"] = clsp.tile([128, 4], F32, tag="tevc", name=f"tevc{b}")
                st["tcorr"] = clsp.tile([128, 4], F32, tag="tcorr", name=f"tcorr{b}")
                nc.vector.memset(oscls, 0.0)
                # zero the kT key-pad region so the chunk-1 S^T matmul can use
                # a full (64,128) shape (paired half-array mode). b0's runs on
                # the idle early DVE; b1's on gpsimd (gpsimd is slow but has
                # ~50us of slack before b1's first pass1).
                eng = nc.vector if b == 0 else nc.gpsimd
                eng.memset(st["kT"][:, :, :, SP1:KPAD], 0.0)

            def emit_xT_chunk(b, ci, eng):
                (t0, tl) = TCH[ci]
                eng.dma_start(
                    out=S[b]["xT"][:, :, t0:t0 + tl],
                    in_=xT[b, :, t0:t0 + tl].rearrange("(c p) t -> p c t", p=128))

            def emit_qk_item(b, oc, ci):
                isq = oc < 4
                hp = oc if isq else oc - 4
                (t0, tl) = TCH[ci]
                ps = ps_proj.tile([128, 512], F32, tag="proj")
                for kc in range(KC):
                    nc.tensor.matmul(
                        ps[:, :tl],
                        lhsT=wqkv_sb[:, kc, oc * 128:(oc + 1) * 128],
                        rhs=S[b]["xT"][:, kc, t0:t0 + tl],
                        start=(kc == 0),
                        stop=(kc == KC - 1),
                    )
                dst = S[b]["qT"] if isq else S[b]["kT"]
                eng = nc.scalar if isq else nc.vector
                cp = eng.copy if isq else eng.tensor_copy
                o0 = 0 if isq else 1
                if ci == 0:
                    cp(
                        dst[:, hp, 0:2, o0:o0 + SP],
                        ps[:, 1:tl].rearrange("p (a s) -> p a s", a=2),
                    )
                    ccol = SP if isq else 0
                    cp(
                        dst[:, hp, 0:F, ccol:ccol + 1],
                        ps[:, None, 0:1].to_broadcast([128, F, 1]),
                    )
                else:
                    cp(
                        dst[:, hp, 2 * ci:2 * ci + 2, o0:o0 + SP],
                        ps[:, :tl].rearrange("p (a s) -> p a s", a=2),
                    )

            def emit_v_item(b, t):
                m = 128 if t < NT - 1 else LAST
                vfl = S[b]["vfl"]
                ps = ps_proj.tile([128, 512], F32, tag="proj")
                for kc in range(KC):
                    nc.tensor.matmul(
                        ps[:m, :],
                        lhsT=S[b]["xT"][:, kc, 128 * t:128 * t + m],
                        rhs=wqkv_sb[:, kc, 2 * INNER:3 * INNER],
                        start=(kc == 0),
                        stop=(kc == KC - 1),
                    )
                if t % 2 == 0:
                    nc.scalar.copy(vfl[:m, t, :], ps[:m, :])
                else:
                    nc.vector.tensor_copy(vfl[:m, t, :], ps[:m, :])
                # frame-aligned rearrangement pieces sourced from this tile,
                # issue slots alternating over the gpsimd + sync queues
                for (kind, f, p0, d0, l) in rearr.get(t, []):
                    dst = S[b]["va"] if kind == "a" else S[b]["vb"]
                    _pc[0] += 1
                    eng = nc.gpsimd if _pc[0] % 2 else nc.sync
                    eng.dma_start(out=dst[d0:d0 + l, f, :], in_=vfl[p0:p0 + l, t, :])
                if t == 0:
                    nc.gpsimd.dma_start(
                        out=S[b]["va"][0:1, 0:F, :],
                        in_=vfl[0:1, 0, None, :].to_broadcast([1, F, INNER]),
                    )
                    pvt = ps_proj.tile([128, 512], F32, tag="proj")
                    for hp in range(4):
                        nc.tensor.matmul(
                            pvt[:, hp:hp + 1],
                            lhsT=vfl[0:1, 0, 128 * hp:128 * (hp + 1)],
                            rhs=ones_row[0:1, 0:1],
                            start=True, stop=True,
                        )
                    nc.vector.tensor_copy(S[b]["vTcls"], pvt[:, 0:4])

            # PSUM accumulation groups landing back-to-back on the SAME bank
            # pay a ~90ns turnaround before the second group can stream; pairs
            # on fresh banks co-run at full rate. The block emitters below
            # interleave the two steps of a block (which use different psum
            # bufs = different banks) so consecutive groups always alternate.
            def p1_block(specs):
                sts = [ps_st.tile([128, 2, 2, 256], F32, tag="st", name=f"st{i}")
                       for i, _ in enumerate(specs)]
                for chunk in (0, 1):
                    c0, c1 = (0, 128) if chunk == 0 else (128, KPAD)
                    for (g, b, hp, f), st_t in zip(specs, sts):
                        qT, kT = S[b]["qT"], S[b]["kT"]
                        for par in range(2):
                            rows = slice(64 * par, 64 * par + 64)
                            nc.tensor.matmul(
                                st_t[:, par, chunk, 0:SP1],
                                lhsT=kT[rows, hp, f, c0:c1],
                                rhs=qT[rows, hp, f, :],
                                start=True, stop=True,
                            )
                for (g, b, hp, f), st_t in zip(specs, sts):
                    ea = ea_ring[:, :, g % EARING, :, :]
                    nc.scalar.activation(ea, st_t[:, :, :, 0:SP1], EXP)
                    if f == 0:
                        for par in range(2):
                            h = 2 * hp + par
                            nc.scalar.copy(S[b]["ecc_row"][0:1, h:h + 1],
                                           ea[0:1, par, 0, SP:SP1])

            def p2_block(specs):
                pos = [ps_po.tile([128, 512], F32, tag="po", name=f"po{i}")
                       for i, _ in enumerate(specs)]
                for phase in range(4):
                    for (g, b, hp, f), po in zip(specs, pos):
                        ea = ea_ring[:, :, g % EARING, :, :]
                        va, vb = S[b]["va"], S[b]["vb"]
                        for par in range(2):
                            rows = slice(64 * par, 64 * par + 64)
                            hs = slice(DH * (2 * hp + par),
                                       DH * (2 * hp + par + 1))
                            if phase == 0:
                                nc.tensor.matmul(
                                    po[rows, 256:256 + SP1],
                                    lhsT=ones128[:, 0:64],
                                    rhs=ea[:, par, 0, :],
                                    start=True, stop=False,
                                )
                            elif phase == 1:
                                nc.tensor.matmul(
                                    po[rows, 256:256 + SP1],
                                    lhsT=ones128[0:69, 0:64],
                                    rhs=ea[0:69, par, 1, :],
                                    start=False, stop=True,
                                )
                            elif phase == 2:
                                nc.tensor.matmul(
                                    po[rows, 0:SP1],
                                    lhsT=va[:, f, hs],
                                    rhs=ea[:, par, 0, :],
                                    start=True, stop=False,
                                )
                            else:
                                nc.tensor.matmul(
                                    po[rows, 0:SP1],
                                    lhsT=vb[0:69, f, hs],
                                    rhs=ea[0:69, par, 1, :],
                                    start=False, stop=True,
                                )
                for i, ((g, b, hp, f), po) in enumerate(zip(specs, pos)):
                    rbc = small.tile([128, SP1], F32, tag="rbc",
                                     name=f"rbc{i}")
                    nc.vector.reciprocal_approx_fast(rbc, po[:, 256:256 + SP1])
                    nc.vector.tensor_tensor(
                        S[b]["attnT"][:, hp, 1 + SP * f:1 + SP * (f + 1)],
                        po[:, 0:SP], rbc[:, 0:SP], MULT,
                    )
                    nc.vector.tensor_tensor(
                        S[b]["oscls"][:, :, hp], po[:, SP:SP + 257:256],
                        S[b]["oscls"][:, :, hp], ADD,
                    )

            def cls_finalize(b):
                st = S[b]
                pec = ps_proj.tile([128, 512], F32, tag="proj")
                for hp in range(4):
                    for par in range(2):
                        h = 2 * hp + par
                        rows = slice(64 * par, 64 * par + 64)
                        nc.tensor.matmul(pec[rows, hp:hp + 1],
                                         lhsT=ones_row[0:1, 0:64],
                                         rhs=st["ecc_row"][0:1, h:h + 1],
                                         start=True, stop=True)
                nc.vector.tensor_copy(st["ecc_bc"], pec[:, 0:4])
                nc.vector.scalar_tensor_tensor(
                    st["scls"], st["ecc_bc"], -7.0, st["scls"], op0=MULT, op1=ADD,
                )
                nc.vector.reciprocal_approx_fast(st["rcls"], st["scls"])
                nc.vector.tensor_tensor(st["tevc"], st["ecc_bc"], st["vTcls"], MULT)
                nc.vector.scalar_tensor_tensor(
                    st["tcorr"], st["tevc"], -7.0, st["ocls"], op0=MULT, op1=ADD,
                )
                nc.vector.tensor_tensor(st["tcorr"], st["tcorr"], st["rcls"], MULT)
                nc.vector.tensor_copy(st["attnT"][:, 0:4, 0:1], st["tcorr"][:, :, None])

            def emit_outproj(b, t):
                m = 128 if t < NT - 1 else LAST
                ps = ps_proj.tile([128, 512], F32, tag="proj")
                act_tail = b == 1 and t in (1, 0)
                for kc in range(KC):
                    nc.tensor.matmul(
                        ps[:m, :],
                        lhsT=S[b]["attnT"][:, kc, 128 * t:128 * t + m],
                        rhs=wout_sb[:, kc, :],
                        start=(kc == 0),
                        stop=(kc == KC - 1) and not act_tail,
                    )
                osb = outp.tile([128, DIM], BF16, tag="out")
                if act_tail:
                    nc.tensor.matmul(
                        ps[:m, :],
                        lhsT=ones_row[0:1, 0:m],
                        rhs=bout_bf[0:1, :],
                        start=False, stop=True,
                    )
                    nc.scalar.copy(osb[:m, :], ps[:m, :])
                else:
                    nc.vector.tensor_tensor(osb[:m, :], ps[:m, :], bout_bc[:m, :], ADD)
                nc.sync.dma_start(out=out[b, 128 * t:128 * t + m, :], in_=osb[:m, :])

            # ---------------- stage A: b0 projection ----------
            steps_l = [(0, hp, f) for hp in range(4) for f in range(F)]
            steps_l += [(1, hp, f) for f in FORDER1 for hp in range(4)]
            alloc_batch(0)
            # input DMAs: one dma_start per logical load; first-needed loads
            # lead their queue and are split so the first v/qk matmuls can
            # start on partial data. scalar: wqkv-v, q-hp0, ci1, ci3; sync: rest.
            nc.scalar.dma_start(out=wqkv_sb[:, 0:2, 2 * INNER:3 * INNER],
                                in_=wqkv[0:256, 2 * INNER:3 * INNER].rearrange("(c p) o -> p c o", p=128))
            nc.scalar.dma_start(out=wqkv_sb[:, 2:4, 2 * INNER:3 * INNER],
                                in_=wqkv[256:512, 2 * INNER:3 * INNER].rearrange("(c p) o -> p c o", p=128))
            nc.sync.dma_start(out=S[0]["xT"][:, :, 0:128],
                              in_=xT[0, :, 0:128].rearrange("(c p) t -> p c t", p=128))
            nc.sync.dma_start(out=S[0]["xT"][:, :, 128:TCH[0][1]],
                              in_=xT[0, :, 128:TCH[0][1]].rearrange("(c p) t -> p c t", p=128))
            load_wqkv_cols(nc.scalar, 0, 128)          # q hp0
            load_wqkv_cols(nc.sync, 512, 640)          # k hp0
            emit_xT_chunk(0, 1, nc.scalar)
            emit_xT_chunk(0, 2, nc.sync)
            emit_xT_chunk(0, 3, nc.scalar)
            load_wqkv_cols(nc.sync, 128, 512)          # q hp1-3
            load_wqkv_cols(nc.sync, 640, 1024)         # k hp1-3
            nc.sync.dma_start(out=wout_sb, in_=wout[:, :].rearrange("(c p) o -> p c o", p=128))
            # bout_bc on the idle gpsimd queue so it lands ~12us: the tile
            # scheduler hoists the bout_bf copy to ~24us wherever we emit it,
            # and a late bout_bc DMA head-of-line blocks the DVE there
            nc.gpsimd.dma_start(out=bout_bc, in_=bout[:, :].to_broadcast([128, DIM]))

            # v/qk ordering follows the DMA arrival order (v-part + ci0 first)
            emit_v_item(0, 0)
            emit_v_item(0, 1)
            emit_qk_item(0, 0, 0)
            emit_qk_item(0, 4, 0)
            emit_v_item(0, 2)
            emit_v_item(0, 3)
            emit_qk_item(0, 0, 1)
            emit_qk_item(0, 4, 1)
            emit_v_item(0, 4)
            emit_v_item(0, 5)
            emit_qk_item(0, 0, 2)
            emit_qk_item(0, 4, 2)
            emit_v_item(0, 6)
            emit_v_item(0, 7)
            emit_qk_item(0, 0, 3)
            emit_qk_item(0, 4, 3)
            # hoist the first two pass1 blocks: their exps enter the ACT
            # queue ahead of the remaining stage-A copies, so the loop's
            # first pass2 blocks don't stall on a ~3us ACT backlog
            p1_block([(0,) + steps_l[0], (1,) + steps_l[1]])
            emit_v_item(0, 8)
            emit_v_item(0, 9)
            p1_block([(2,) + steps_l[2], (3,) + steps_l[3]])
            for t in range(10, NT):
                emit_v_item(0, t)

            # ---------------- main loop schedule ----------------
            sched = {}

            def add(g, fn, *a):
                sched.setdefault(g, []).append((fn, a))

            # b0 qk items hp1-3: 2 per iteration over g 0..11
            cnt = 0
            for hp in (1, 2, 3):
                for ci in range(4):
                    for oc in (hp, hp + 4):
                        add(cnt // 2, emit_qk_item, 0, oc, ci)
                        cnt += 1

            # b1: alloc + xT chunks on sync
            add(2, alloc_batch, 1)
            for ci in range(4):
                add(2 + 2 * ci, emit_xT_chunk, 1, ci, nc.sync)
            # b1 V items
            for t in range(NT):
                add(8 + t, emit_v_item, 1, t)
            # b1 qk, ci-major (frame order needs ci0 first, then ci1..3)
            cnt = 0
            for ci in range(4):
                for hp in range(4):
                    for oc in (hp, hp + 4):
                        add(14 + cnt // 2, emit_qk_item, 1, oc, ci)
                        cnt += 1

            # bout_bf staging for the act_tail bias matmuls (needed ~g66).
            # Late placement: the DVE queue must not reach this before the
            # bout_bc DMA lands (~25us) or it head-of-line blocks the kT casts.
            add(40, nc.vector.tensor_copy, bout_bf, bout_bc[0:1, :])

            # b0 outproj: hp3 step for frame f is at g = 24+f, pass2 at 26+f;
            # +1 extra bucket of slack so the PE doesn't catch the DVE attnT
            # writes mid-flight
            def fmax0(t):
                return min(F - 1, (128 * (t + 1) - 2) // SP)
            add(34, cls_finalize, 0)
            for t in range(1, NT):
                add(29 + fmax0(t), emit_outproj, 0, t)
            add(37, emit_outproj, 0, 0)

            # b1 outproj: last hp step of frame at position k is g = G1+4k+3
            fpos1 = {f: k for k, f in enumerate(FORDER1)}
            for t in range(2, NT):
                # frames touched by tile t: fmin..fmax
                fmin = max(0, (128 * t - 1) // SP)
                fmx = fmax0(t)
                kmax = max(fpos1[f] for f in range(fmin, fmx + 1))
                add(min(62, G1 + 4 * kmax + 3 + LAG + 3), emit_outproj, 1, t)
            add(NSTEP_L + LAG, cls_finalize, 1)
            add(NSTEP_L + LAG, emit_outproj, 1, 1)
            add(NSTEP_L + LAG + 1, emit_outproj, 1, 0)

            # block-2 iteration: two attention steps run back-to-back (24
            # contiguous PE matmuls) before the projection items interleave;
            # mixing proj items between single steps measured ~35ns/mm slower.
            # EARING=4 exactly covers the 4 live exp slots this creates.
            for gb in range(0, NSTEP_L + LAG + 2, 2):
                items0 = sched.pop(gb, [])
                items1 = sched.pop(gb + 1, [])
                if items0:
                    fn, a = items0.pop(0)
                    fn(*a)
                p1specs = [(g,) + steps_l[g] for g in (gb, gb + 1)
                           if 4 <= g < NSTEP_L]
                if p1specs:
                    p1_block(p1specs)
                p2specs = [(g - LAG,) + steps_l[g - LAG] for g in (gb, gb + 1)
                           if g >= LAG and g - LAG < NSTEP_L]
                if p2specs:
                    p2_block(p2specs)
                for (fn, a) in items0 + items1:
                    fn(*a)
            for g in sorted(sched):
                for (fn, a) in sched[g]:
                    fn(*a)

    nc.finalize()
    return nc


_CACHE = {}


def _get_nc():
    if "nc" not in _CACHE:
        _CACHE["nc"] = build_nc()
    return _CACHE["nc"]


def prepare_in_maps(x, f, W_qkv, W_out, b_out):
    assert int(f) == F
    x = np.asarray(x, dtype=np.float32)
    W_qkv = np.asarray(W_qkv, dtype=np.float32).copy()
    W_out = np.asarray(W_out, dtype=np.float32)
    b_out = np.asarray(b_out, dtype=np.float32)
    W_qkv[:, :INNER] *= DH ** -0.5
    wqkv_bf = W_qkv.astype(NPBF)
    wout_bf = W_out.astype(NPBF)
    bout_np = b_out.reshape(1, DIM)
    xT = np.ascontiguousarray(x.transpose(0, 2, 1)).astype(NPBF)
    in_maps = []
    for c in range(NCORES):
        in_maps.append({
            "xT": np.ascontiguousarray(xT[NB * c:NB * (c + 1)]),
            "wqkv": wqkv_bf,
            "wout": wout_bf,
            "bout": bout_np,
        })
    return in_maps


def kernel(x, f, W_qkv, W_out, b_out):
    nc = _get_nc()
    in_maps = prepare_in_maps(x, f, W_qkv, W_out, b_out)
    res = bass_utils.run_bass_kernel_spmd(nc, in_maps, list(range(NCORES)))
    return np.concatenate(
        [np.asarray(r["out"], dtype=np.float32) for r in res.results], axis=0)


# revision 30
# speedup vs baseline: 1.1357x; 1.1357x over previous
"""DividedAttention (TimeSformer-style divided space-time attention) on 8 trn2 cores.

Sharding: pure data-parallel over batch B=16 -> 2 batch items per core.

v9 (final), from v2's two-batch pipeline. Measured ~175-177us vs 207us for
v2 under the same harness. Changes, in decreasing order of impact:
  - PSUM bank alternation (-12us): back-to-back accumulation groups on the
    same psum bank stall ~90ns before the second group streams. Attention
    is emitted in blocks of two steps with the two steps' matmul groups
    interleaved (different pool bufs = different banks), so consecutive
    groups always land on alternating banks.
  - block-2 attention (-3us): both steps' 24 attention matmuls run
    back-to-back before projection items interleave; mixed single-step
    emission measured ~35ns/mm slower.
  - kT padded to 128 key columns per chunk (-6us): the S^T chunk-1 pair
    becomes (64,128)-shaped; a 65..127-partition output breaks the PE's
    paired half-array mode (211ns vs 110ns per pair).
  - consolidated input DMAs + queue split (-8us startup): one dma_start
    per xT token-chunk (a dma_start costs ~850ns of issue time on its
    host engine), first-needed slices (wqkv-v kc01, xT tokens 0:128,
    hp0 q/k) lead the scalar/sync queues; va/vb rearrangement pieces
    alternate gpsimd+sync so neither queue serializes.
  - b1 frame order [1..7,0] + ready-timed outproj emission: only out-tile
    t=1 and the cls tile t=0 depend on the last attention step.
  - out stores in bf16 (halves output DMA; host converts back to f32).
  - first two pass1 blocks hoisted into stage A so their exps precede the
    remaining stage-A ACT copies (loop-seam stall).
Known remaining: ~3us DVE stall at the stage-A/loop seam (the scheduler
hoists the bout_bf copy to ~24us where it waits the bout_bc DMA; moving
the DMA to gpsimd made the broadcast pathological, 206us - do not),
~4us of ~470ns outproj-waits-DVE gaps near the tail, ~5us fixed epilogue.
"""
import sys

sys.path.insert(0, "/opt/trn_rl_repo")

import numpy as np
import ml_dtypes

from concourse import bacc
import concourse.mybir as mybir
import concourse.tile as tile
from concourse import bass_utils

BF16 = mybir.dt.bfloat16
F32 = mybir.dt.float32
NPBF = ml_dtypes.bfloat16

B, SP, F, DIM, H, DH = 16, 196, 8, 512, 8, 64
INNER = H * DH            # 512
N = 1 + F * SP            # 1569
SP1 = SP + 1              # 197
KPAD = 256                # kT padded key columns (2 chunks of 128)
NCORES = 8
NB = B // NCORES          # 2
KC = DIM // 128           # 4
NT = (N + 127) // 128     # 13
LAST = N - 128 * (NT - 1)  # 33
TCH = [(0, 1 + 2 * SP), (1 + 2 * SP, 2 * SP), (1 + 4 * SP, 2 * SP), (1 + 6 * SP, 2 * SP)]

EXP = mybir.ActivationFunctionType.Exp
ADD = mybir.AluOpType.add
MULT = mybir.AluOpType.mult

LAG = 2
EARING = 4
FORDER1 = [1, 2, 3, 4, 5, 6, 7, 0]   # b1 frame order; f0 last -> tail is t1+t0 only
G1 = 32                              # loop iteration where b1 steps start
NSTEP_L = 32 + 32                    # b0 + b1


def _v_pieces(tok0, length):
    out = []
    done = 0
    while done < length:
        tok = tok0 + done
        t, p0 = divmod(tok, 128)
        l = min(128 - p0, length - done)
        out.append((t, p0, done, l))
        done += l
    return out


def _rearr_by_tile():
    """v_flat tile -> list of (dst_kind, frame, src_p0, dst_p0, len)."""
    by_tile = {}
    for f in range(F):
        for (t, p0, d0, l) in _v_pieces(1 + SP * f, 127):
            by_tile.setdefault(t, []).append(("a", f, p0, 1 + d0, l))
        for (t, p0, d0, l) in _v_pieces(128 + SP * f, 69):
            by_tile.setdefault(t, []).append(("b", f, p0, d0, l))
    return by_tile


def build_nc():
    nc = bacc.Bacc(num_devices=NCORES)

    xT = nc.declare_dram_parameter("xT", [NB, DIM, N], BF16, isOutput=False)
    wqkv = nc.declare_dram_parameter("wqkv", [DIM, 3 * INNER], BF16, isOutput=False)
    wout = nc.declare_dram_parameter("wout", [INNER, DIM], BF16, isOutput=False)
    bout = nc.declare_dram_parameter("bout", [1, DIM], F32, isOutput=False)
    out = nc.declare_dram_parameter("out", [NB, N, DIM], BF16, isOutput=True)

    rearr = _rearr_by_tile()

    with tile.TileContext(nc) as tc:
        with (
            tc.tile_pool(name="const", bufs=1) as const,
            tc.tile_pool(name="perb", bufs=2) as perb,
            tc.tile_pool(name="vflat", bufs=1) as vflat_pool,
            tc.tile_pool(name="clsp", bufs=2) as clsp,
            tc.tile_pool(name="small", bufs=3) as small,
            tc.tile_pool(name="outp", bufs=3) as outp,
            tc.tile_pool(name="ps_proj", bufs=2, space="PSUM") as ps_proj,
            tc.tile_pool(name="ps_st", bufs=2, space="PSUM") as ps_st,
            tc.tile_pool(name="ps_po", bufs=2, space="PSUM") as ps_po,
        ):
            # ---- constants (memory only; DMAs are emitted in the schedule)
            wqkv_sb = const.tile([128, KC, 3 * INNER], BF16)
            wout_sb = const.tile([128, KC, DIM], BF16)
            ones128 = const.tile([128, 64], BF16)
            nc.vector.memset(ones128, 1.0)
            ones_row = const.tile([1, 128], BF16)
            nc.vector.memset(ones_row, 1.0)
            ea_ring = const.tile([128, 2, EARING, 2, SP1], BF16)
            bout_bc = const.tile([128, DIM], F32)
            bout_bf = const.tile([1, DIM], BF16)
            nc.vector.memset(bout_bf, 0.0)

            def load_wqkv_cols(eng, c0, c1):
                eng.dma_start(
                    out=wqkv_sb[:, :, c0:c1],
                    in_=wqkv[:, c0:c1].rearrange("(c p) o -> p c o", p=128))

            S = [dict() for _ in range(NB)]
            _pc = [0]

            def alloc_batch(b):
                st = S[b]
                st["xT"] = perb.tile([128, KC, N], BF16, tag="xt", name=f"xt{b}")
                st["qT"] = perb.tile([128, 4, F, SP1], BF16, tag="qT", name=f"qT{b}")
                st["kT"] = perb.tile([128, 4, F, KPAD], BF16, tag="kT", name=f"kT{b}")
                st["vfl"] = vflat_pool.tile([128, NT, INNER], BF16, tag="vfl", name=f"vfl{b}")
                st["va"] = perb.tile([128, F, INNER], BF16, tag="vfra", name=f"vfra{b}")
                st["vb"] = perb.tile([128, F, INNER], BF16, tag="vfrb", name=f"vfrb{b}")
                st["attnT"] = perb.tile([128, KC, N], BF16, tag="attnT", name=f"attnT{b}")
                oscls = clsp.tile([128, 2, 4], F32, tag="oscls", name=f"oscls{b}")
                st["oscls"] = oscls
                st["ocls"] = oscls[:, 0, :]
                st["scls"] = oscls[:, 1, :]
                st["ecc_row"] = clsp.tile([1, H], BF16, tag="eccrow", name=f"eccrow{b}")
                st["vTcls"] = clsp.tile([128, 4], F32, tag="vTcls", name=f"vTcls{b}")
                st["ecc_bc"] = clsp.tile([128, 4], F32, tag="eccbc", name=f"eccbc{b}")
                st["rcls"] = clsp.tile([128, 4], F32, tag="rcls", name=f"rcls{b}")
                st["tevc"] = clsp.tile([128, 4], F32, tag="tevc", name=f"tevc{b}")
                st["tcorr"] = clsp.tile([128, 4], F32, tag="tcorr", name=f"tcorr{b}")
                nc.vector.memset(oscls, 0.0)
                # zero the kT key-pad region so the chunk-1 S^T matmul can use
                # a full (64,128) shape (paired half-array mode). b0's runs on
                # the idle early DVE; b1's on gpsimd (gpsimd is slow but has
                # ~50us of slack before b1's first pass1).
                eng = nc.vector if b == 0 else nc.gpsimd
                eng.memset(st["kT"][:, :, :, SP1:KPAD], 0.0)

            def emit_xT_chunk(b, ci, eng):
                (t0, tl) = TCH[ci]
                eng.dma_start(
                    out=S[b]["xT"][:, :, t0:t0 + tl],
                    in_=xT[b, :, t0:t0 + tl].rearrange("(c p) t -> p c t", p=128))

            def emit_qk_item(b, oc, ci):
                isq = oc < 4
                hp = oc if isq else oc - 4
                (t0, tl) = TCH[ci]
                ps = ps_proj.tile([128, 512], F32, tag="proj")
                for kc in range(KC):
                    nc.tensor.matmul(
                        ps[:, :tl],
                        lhsT=wqkv_sb[:, kc, oc * 128:(oc + 1) * 128],
                        rhs=S[b]["xT"][:, kc, t0:t0 + tl],
                        start=(kc == 0),
                        stop=(kc == KC - 1),
                    )
                dst = S[b]["qT"] if isq else S[b]["kT"]
                eng = nc.scalar if isq else nc.vector
                cp = eng.copy if isq else eng.tensor_copy
                o0 = 0 if isq else 1
                if ci == 0:
                    cp(
                        dst[:, hp, 0:2, o0:o0 + SP],
                        ps[:, 1:tl].rearrange("p (a s) -> p a s", a=2),
                    )
                    ccol = SP if isq else 0
                    cp(
                        dst[:, hp, 0:F, ccol:ccol + 1],
                        ps[:, None, 0:1].to_broadcast([128, F, 1]),
                    )
                else:
                    cp(
                        dst[:, hp, 2 * ci:2 * ci + 2, o0:o0 + SP],
                        ps[:, :tl].rearrange("p (a s) -> p a s", a=2),
                    )

            def emit_v_item(b, t):
                m = 128 if t < NT - 1 else LAST
                vfl = S[b]["vfl"]
                ps = ps_proj.tile([128, 512], F32, tag="proj")
                for kc in range(KC):
                    nc.tensor.matmul(
                        ps[:m, :],
                        lhsT=S[b]["xT"][:, kc, 128 * t:128 * t + m],
                        rhs=wqkv_sb[:, kc, 2 * INNER:3 * INNER],
                        start=(kc == 0),
                        stop=(kc == KC - 1),
                    )
                if t % 2 == 0:
                    nc.scalar.copy(vfl[:m, t, :], ps[:m, :])
                else:
                    nc.vector.tensor_copy(vfl[:m, t, :], ps[:m, :])
                # frame-aligned rearrangement pieces sourced from this tile,
                # issue slots alternating over the gpsimd + sync queues
                for (kind, f, p0, d0, l) in rearr.get(t, []):
                    dst = S[b]["va"] if kind == "a" else S[b]["vb"]
                    _pc[0] += 1
                    eng = nc.gpsimd if _pc[0] % 2 else nc.sync
                    eng.dma_start(out=dst[d0:d0 + l, f, :], in_=vfl[p0:p0 + l, t, :])
                if t == 0:
                    nc.gpsimd.dma_start(
                        out=S[b]["va"][0:1, 0:F, :],
                        in_=vfl[0:1, 0, None, :].to_broadcast([1, F, INNER]),
                    )
                    pvt = ps_proj.tile([128, 512], F32, tag="proj")
                    for hp in range(4):
                        nc.tensor.matmul(
                            pvt[:, hp:hp + 1],
                            lhsT=vfl[0:1, 0, 128 * hp:128 * (hp + 1)],
                            rhs=ones_row[0:1, 0:1],
                            start=True, stop=True,
                        )
                    nc.vector.tensor_copy(S[b]["vTcls"], pvt[:, 0:4])

            # PSUM accumulation groups landing back-to-back on the SAME bank
            # pay a ~90ns turnaround before the second group can stream; pairs
            # on fresh banks co-run at full rate. The block emitters below
            # interleave the two steps of a block (which use different psum
            # bufs = different banks) so consecutive groups always alternate.
            def p1_block(specs):
                sts = [ps_st.tile([128, 2, 2, 256], F32, tag="st", name=f"st{i}")
                       for i, _ in enumerate(specs)]
                for chunk in (0, 1):
                    c0, c1 = (0, 128) if chunk == 0 else (128, KPAD)
                    for (g, b, hp, f), st_t in zip(specs, sts):
                        qT, kT = S[b]["qT"], S[b]["kT"]
                        for par in range(2):
                            rows = slice(64 * par, 64 * par + 64)
                            nc.tensor.matmul(
                                st_t[:, par, chunk, 0:SP1],
                                lhsT=kT[rows, hp, f, c0:c1],
                                rhs=qT[rows, hp, f, :],
                                start=True, stop=True,
                            )
                for (g, b, hp, f), st_t in zip(specs, sts):
                    ea = ea_ring[:, :, g % EARING, :, :]
                    nc.scalar.activation(ea, st_t[:, :, :, 0:SP1], EXP)
                    if f == 0:
                        for par in range(2):
                            h = 2 * hp + par
                            nc.scalar.copy(S[b]["ecc_row"][0:1, h:h + 1],
                                           ea[0:1, par, 0, SP:SP1])

            def p2_block(specs):
                pos = [ps_po.tile([128, 512], F32, tag="po", name=f"po{i}")
                       for i, _ in enumerate(specs)]
                for phase in range(4):
                    for (g, b, hp, f), po in zip(specs, pos):
                        ea = ea_ring[:, :, g % EARING, :, :]
                        va, vb = S[b]["va"], S[b]["vb"]
                        for par in range(2):
                            rows = slice(64 * par, 64 * par + 64)
                            hs = slice(DH * (2 * hp + par),
                                       DH * (2 * hp + par + 1))
                            if phase == 0:
                                nc.tensor.matmul(
                                    po[rows, 256:256 + SP1],
                                    lhsT=ones128[:, 0:64],
                                    rhs=ea[:, par, 0, :],
                                    start=True, stop=False,
                                )
                            elif phase == 1:
                                nc.tensor.matmul(
                                    po[rows, 256:256 + SP1],
                                    lhsT=ones128[0:69, 0:64],
                                    rhs=ea[0:69, par, 1, :],
                                    start=False, stop=True,
                                )
                            elif phase == 2:
                                nc.tensor.matmul(
                                    po[rows, 0:SP1],
                                    lhsT=va[:, f, hs],
                                    rhs=ea[:, par, 0, :],
                                    start=True, stop=False,
                                )
                            else:
                                nc.tensor.matmul(
                                    po[rows, 0:SP1],
                                    lhsT=vb[0:69, f, hs],
                                    rhs=ea[0:69, par, 1, :],
                                    start=False, stop=True,
                                )
                for i, ((g, b, hp, f), po) in enumerate(zip(specs, pos)):
                    rbc = small.tile([128, SP1], F32, tag="rbc",
                                     name=f"rbc{i}")
                    nc.vector.reciprocal_approx_fast(rbc, po[:, 256:256 + SP1])
                    nc.vector.tensor_tensor(
                        S[b]["attnT"][:, hp, 1 + SP * f:1 + SP * (f + 1)],
                        po[:, 0:SP], rbc[:, 0:SP], MULT,
                    )
                    nc.vector.tensor_tensor(
                        S[b]["oscls"][:, :, hp], po[:, SP:SP + 257:256],
                        S[b]["oscls"][:, :, hp], ADD,
                    )

            def cls_finalize(b):
                st = S[b]
                pec = ps_proj.tile([128, 512], F32, tag="proj")
                for hp in range(4):
                    for par in range(2):
                        h = 2 * hp + par
                        rows = slice(64 * par, 64 * par + 64)
                        nc.tensor.matmul(pec[rows, hp:hp + 1],
                                         lhsT=ones_row[0:1, 0:64],
                                         rhs=st["ecc_row"][0:1, h:h + 1],
                                         start=True, stop=True)
                nc.vector.tensor_copy(st["ecc_bc"], pec[:, 0:4])
                nc.vector.scalar_tensor_tensor(
                    st["scls"], st["ecc_bc"], -7.0, st["scls"], op0=MULT, op1=ADD,
                )
                nc.vector.reciprocal_approx_fast(st["rcls"], st["scls"])
                nc.vector.tensor_tensor(st["tevc"], st["ecc_bc"], st["vTcls"], MULT)
                nc.vector.scalar_tensor_tensor(
                    st["tcorr"], st["tevc"], -7.0, st["ocls"], op0=MULT, op1=ADD,
                )
                nc.vector.tensor_tensor(st["tcorr"], st["tcorr"], st["rcls"], MULT)
                nc.vector.tensor_copy(st["attnT"][:, 0:4, 0:1], st["tcorr"][:, :, None])

            def emit_outproj(b, t):
                m = 128 if t < NT - 1 else LAST
                ps = ps_proj.tile([128, 512], F32, tag="proj")
                act_tail = b == 1 and t in (1, 0)
                for kc in range(KC):
                    nc.tensor.matmul(
                        ps[:m, :],
                        lhsT=S[b]["attnT"][:, kc, 128 * t:128 * t + m],
                        rhs=wout_sb[:, kc, :],
                        start=(kc == 0),
                        stop=(kc == KC - 1) and not act_tail,
                    )
                osb = outp.tile([128, DIM], BF16, tag="out")
                if act_tail:
                    nc.tensor.matmul(
                        ps[:m, :],
                        lhsT=ones_row[0:1, 0:m],
                        rhs=bout_bf[0:1, :],
                        start=False, stop=True,
                    )
                    nc.scalar.copy(osb[:m, :], ps[:m, :])
                else:
                    nc.vector.tensor_tensor(osb[:m, :], ps[:m, :], bout_bc[:m, :], ADD)
                nc.sync.dma_start(out=out[b, 128 * t:128 * t + m, :], in_=osb[:m, :])

            # ---------------- stage A: b0 projection ----------
            steps_l = [(0, hp, f) for hp in range(4) for f in range(F)]
            steps_l += [(1, hp, f) for f in FORDER1 for hp in range(4)]
            alloc_batch(0)
            # input DMAs: one dma_start per logical load; first-needed loads
            # lead their queue and are split so the first v/qk matmuls can
            # start on partial data. scalar: wqkv-v, q-hp0, ci1, ci3; sync: rest.
            nc.scalar.dma_start(out=wqkv_sb[:, 0:2, 2 * INNER:3 * INNER],
                                in_=wqkv[0:256, 2 * INNER:3 * INNER].rearrange("(c p) o -> p c o", p=128))
            nc.scalar.dma_start(out=wqkv_sb[:, 2:4, 2 * INNER:3 * INNER],
                                in_=wqkv[256:512, 2 * INNER:3 * INNER].rearrange("(c p) o -> p c o", p=128))
            nc.sync.dma_start(out=S[0]["xT"][:, :, 0:128],
                              in_=xT[0, :, 0:128].rearrange("(c p) t -> p c t", p=128))
            nc.sync.dma_start(out=S[0]["xT"][:, :, 128:TCH[0][1]],
                              in_=xT[0, :, 128:TCH[0][1]].rearrange("(c p) t -> p c t", p=128))
            load_wqkv_cols(nc.scalar, 0, 128)          # q hp0
            load_wqkv_cols(nc.sync, 512, 640)          # k hp0
            emit_xT_chunk(0, 1, nc.scalar)
            emit_xT_chunk(0, 2, nc.sync)
            emit_xT_chunk(0, 3, nc.scalar)
            load_wqkv_cols(nc.sync, 128, 512)          # q hp1-3
            load_wqkv_cols(nc.sync, 640, 1024)         # k hp1-3
            nc.sync.dma_start(out=wout_sb, in_=wout[:, :].rearrange("(c p) o -> p c o", p=128))
            nc.sync.dma_start(out=bout_bc, in_=bout[:, :].to_broadcast([128, DIM]))

            # v/qk ordering follows the DMA arrival order (v-part + ci0 first)
            emit_v_item(0, 0)
            emit_v_item(0, 1)
            emit_qk_item(0, 0, 0)
            emit_qk_item(0, 4, 0)
            emit_v_item(0, 2)
            emit_v_item(0, 3)
            emit_qk_item(0, 0, 1)
            emit_qk_item(0, 4, 1)
            emit_v_item(0, 4)
            emit_v_item(0, 5)
            emit_qk_item(0, 0, 2)
            emit_qk_item(0, 4, 2)
            emit_v_item(0, 6)
            emit_v_item(0, 7)
            emit_qk_item(0, 0, 3)
            emit_qk_item(0, 4, 3)
            # hoist the first two pass1 blocks: their exps enter the ACT
            # queue ahead of the remaining stage-A copies, so the loop's
            # first pass2 blocks don't stall on a ~3us ACT backlog
            p1_block([(0,) + steps_l[0], (1,) + steps_l[1]])
            emit_v_item(0, 8)
            emit_v_item(0, 9)
            p1_block([(2,) + steps_l[2], (3,) + steps_l[3]])
            for t in range(10, NT):
                emit_v_item(0, t)

            # ---------------- main loop schedule ----------------
            sched = {}

            def add(g, fn, *a):
                sched.setdefault(g, []).append((fn, a))

            # b0 qk items hp1-3: 2 per iteration over g 0..11
            cnt = 0
            for hp in (1, 2, 3):
                for ci in range(4):
                    for oc in (hp, hp + 4):
                        add(cnt // 2, emit_qk_item, 0, oc, ci)
                        cnt += 1

            # b1: alloc + xT chunks on sync
            add(2, alloc_batch, 1)
            for ci in range(4):
                add(2 + 2 * ci, emit_xT_chunk, 1, ci, nc.sync)
            # b1 V items
            for t in range(NT):
                add(8 + t, emit_v_item, 1, t)
            # b1 qk, ci-major (frame order needs ci0 first, then ci1..3)
            cnt = 0
            for ci in range(4):
                for hp in range(4):
                    for oc in (hp, hp + 4):
                        add(14 + cnt // 2, emit_qk_item, 1, oc, ci)
                        cnt += 1

            # bout_bf staging for the act_tail bias matmuls (needed ~g66).
            # Late placement: the DVE queue must not reach this before the
            # bout_bc DMA lands (~25us) or it head-of-line blocks the kT casts.
            add(40, nc.vector.tensor_copy, bout_bf, bout_bc[0:1, :])

            # b0 outproj: hp3 step for frame f is at g = 24+f, pass2 at 26+f;
            # +1 extra bucket of slack so the PE doesn't catch the DVE attnT
            # writes mid-flight
            def fmax0(t):
                return min(F - 1, (128 * (t + 1) - 2) // SP)
            add(34, cls_finalize, 0)
            for t in range(1, NT):
                add(29 + fmax0(t), emit_outproj, 0, t)
            add(37, emit_outproj, 0, 0)

            # b1 outproj: last hp step of frame at position k is g = G1+4k+3
            fpos1 = {f: k for k, f in enumerate(FORDER1)}
            for t in range(2, NT):
                # frames touched by tile t: fmin..fmax
                fmin = max(0, (128 * t - 1) // SP)
                fmx = fmax0(t)
                kmax = max(fpos1[f] for f in range(fmin, fmx + 1))
                add(min(62, G1 + 4 * kmax + 3 + LAG + 3), emit_outproj, 1, t)
            add(NSTEP_L + LAG, cls_finalize, 1)
            add(NSTEP_L + LAG, emit_outproj, 1, 1)
            add(NSTEP_L + LAG + 1, emit_outproj, 1, 0)

            # block-2 iteration: two attention steps run back-to-back (24
            # contiguous PE matmuls) before the projection items interleave;
            # mixing proj items between single steps measured ~35ns/mm slower.
            # EARING=4 exactly covers the 4 live exp slots this creates.
            for gb in range(0, NSTEP_L + LAG + 2, 2):
                items0 = sched.pop(gb, [])
                items1 = sched.pop(gb + 1, [])
                if items0:
                    fn, a = items0.pop(0)
                    fn(*a)
                p1specs = [(g,) + steps_l[g] for g in (gb, gb + 1)
                           if 4 <= g < NSTEP_L]
                if p1specs:
                    p1_block(p1specs)
                p2specs = [(g - LAG,) + steps_l[g - LAG] for g in (gb, gb + 1)
                           if g >= LAG and g - LAG < NSTEP_L]
                if p2specs:
                    p2_block(p2specs)
                for (fn, a) in items0 + items1:
                    fn(*a)
            for g in sorted(sched):
                for (fn, a) in sched[g]:
                    fn(*a)

    nc.finalize()
    return nc


_CACHE = {}


def _get_nc():
    if "nc" not in _CACHE:
        _CACHE["nc"] = build_nc()
    return _CACHE["nc"]


def prepare_in_maps(x, f, W_qkv, W_out, b_out):
    assert int(f) == F
    x = np.asarray(x, dtype=np.float32)
    W_qkv = np.asarray(W_qkv, dtype=np.float32).copy()
    W_out = np.asarray(W_out, dtype=np.float32)
    b_out = np.asarray(b_out, dtype=np.float32)
    W_qkv[:, :INNER] *= DH ** -0.5
    wqkv_bf = W_qkv.astype(NPBF)
    wout_bf = W_out.astype(NPBF)
    bout_np = b_out.reshape(1, DIM)
    xT = np.ascontiguousarray(x.transpose(0, 2, 1)).astype(NPBF)
    in_maps = []
    for c in range(NCORES):
        in_maps.append({
            "xT": np.ascontiguousarray(xT[NB * c:NB * (c + 1)]),
            "wqkv": wqkv_bf,
            "wout": wout_bf,
            "bout": bout_np,
        })
    return in_maps


def kernel(x, f, W_qkv, W_out, b_out):
    nc = _get_nc()
    in_maps = prepare_in_maps(x, f, W_qkv, W_out, b_out)
    res = bass_utils.run_bass_kernel_spmd(nc, in_maps, list(range(NCORES)))
    return np.concatenate(
        [np.asarray(r["out"], dtype=np.float32) for r in res.results], axis=0)


# revision 33
# speedup vs baseline: 1.1966x; 1.0536x over previous
"""DividedAttention (TimeSformer-style divided space-time attention) on 8 trn2 cores.

Sharding: pure data-parallel over batch B=16 -> 2 batch items per core.

v9 (final), from v2's two-batch pipeline. Measured ~175-177us vs 207us for
v2 under the same harness. Changes, in decreasing order of impact:
  - PSUM bank alternation (-12us): back-to-back accumulation groups on the
    same psum bank stall ~90ns before the second group streams. Attention
    is emitted in blocks of two steps with the two steps' matmul groups
    interleaved (different pool bufs = different banks), so consecutive
    groups always land on alternating banks.
  - block-2 attention (-3us): both steps' 24 attention matmuls run
    back-to-back before projection items interleave; mixed single-step
    emission measured ~35ns/mm slower.
  - kT padded to 128 key columns per chunk (-6us): the S^T chunk-1 pair
    becomes (64,128)-shaped; a 65..127-partition output breaks the PE's
    paired half-array mode (211ns vs 110ns per pair).
  - consolidated input DMAs + queue split (-8us startup): one dma_start
    per xT token-chunk (a dma_start costs ~850ns of issue time on its
    host engine), first-needed slices (wqkv-v kc01, xT tokens 0:128,
    hp0 q/k) lead the scalar/sync queues; va/vb rearrangement pieces
    alternate gpsimd+sync so neither queue serializes.
  - b1 frame order [1..7,0] + ready-timed outproj emission: only out-tile
    t=1 and the cls tile t=0 depend on the last attention step.
  - out stores in bf16 (halves output DMA; host converts back to f32).
  - first two pass1 blocks hoisted into stage A so their exps precede the
    remaining stage-A ACT copies (loop-seam stall).
Known remaining: ~3us DVE stall at the stage-A/loop seam (the scheduler
hoists the bout_bf copy to ~24us where it waits the bout_bc DMA; moving
the DMA to gpsimd made the broadcast pathological, 206us - do not),
~4us of ~470ns outproj-waits-DVE gaps near the tail, ~5us fixed epilogue.
"""
import sys

sys.path.insert(0, "/opt/trn_rl_repo")

import numpy as np
import ml_dtypes

from concourse import bacc
import concourse.mybir as mybir
import concourse.tile as tile
from concourse import bass_utils

BF16 = mybir.dt.bfloat16
F32 = mybir.dt.float32
NPBF = ml_dtypes.bfloat16

B, SP, F, DIM, H, DH = 16, 196, 8, 512, 8, 64
INNER = H * DH            # 512
N = 1 + F * SP            # 1569
SP1 = SP + 1              # 197
KPAD = 256                # kT padded key columns (2 chunks of 128)
NCORES = 8
NB = B // NCORES          # 2
KC = DIM // 128           # 4
NT = (N + 127) // 128     # 13
LAST = N - 128 * (NT - 1)  # 33
TCH = [(0, 1 + 2 * SP), (1 + 2 * SP, 2 * SP), (1 + 4 * SP, 2 * SP), (1 + 6 * SP, 2 * SP)]

EXP = mybir.ActivationFunctionType.Exp
ADD = mybir.AluOpType.add
MULT = mybir.AluOpType.mult

LAG = 2
EARING = 4
FORDER1 = [1, 2, 3, 4, 5, 6, 7, 0]   # b1 frame order; f0 last -> tail is t1+t0 only
G1 = 32                              # loop iteration where b1 steps start
NSTEP_L = 32 + 32                    # b0 + b1


def _v_pieces(tok0, length):
    out = []
    done = 0
    while done < length:
        tok = tok0 + done
        t, p0 = divmod(tok, 128)
        l = min(128 - p0, length - done)
        out.append((t, p0, done, l))
        done += l
    return out


def _rearr_by_tile():
    """v_flat tile -> list of (dst_kind, frame, src_p0, dst_p0, len)."""
    by_tile = {}
    for f in range(F):
        for (t, p0, d0, l) in _v_pieces(1 + SP * f, 127):
            by_tile.setdefault(t, []).append(("a", f, p0, 1 + d0, l))
        for (t, p0, d0, l) in _v_pieces(128 + SP * f, 69):
            by_tile.setdefault(t, []).append(("b", f, p0, d0, l))
    return by_tile


def build_nc():
    nc = bacc.Bacc(num_devices=NCORES)

    xT = nc.declare_dram_parameter("xT", [NB, DIM, N], BF16, isOutput=False)
    wqkv = nc.declare_dram_parameter("wqkv", [DIM, 3 * INNER], BF16, isOutput=False)
    wout = nc.declare_dram_parameter("wout", [INNER, DIM], BF16, isOutput=False)
    bout = nc.declare_dram_parameter("bout", [1, DIM], F32, isOutput=False)
    out = nc.declare_dram_parameter("out", [NB, N, DIM], BF16, isOutput=True)

    rearr = _rearr_by_tile()

    with tile.TileContext(nc) as tc:
        with (
            tc.tile_pool(name="const", bufs=1) as const,
            tc.tile_pool(name="perb", bufs=2) as perb,
            tc.tile_pool(name="vflat", bufs=1) as vflat_pool,
            tc.tile_pool(name="clsp", bufs=2) as clsp,
            tc.tile_pool(name="small", bufs=3) as small,
            tc.tile_pool(name="outp", bufs=3) as outp,
            tc.tile_pool(name="ps_proj", bufs=2, space="PSUM") as ps_proj,
            tc.tile_pool(name="ps_st", bufs=2, space="PSUM") as ps_st,
            tc.tile_pool(name="ps_po", bufs=2, space="PSUM") as ps_po,
        ):
            # ---- constants (memory only; DMAs are emitted in the schedule)
            wqkv_sb = const.tile([128, KC, 3 * INNER], BF16)
            wout_sb = const.tile([128, KC, DIM], BF16)
            ones128 = const.tile([128, 64], BF16)
            nc.vector.memset(ones128, 1.0)
            ones_row = const.tile([1, 128], BF16)
            nc.vector.memset(ones_row, 1.0)
            ea_ring = const.tile([128, 2, EARING, 2, SP1], BF16)
            bout_bc = const.tile([128, DIM], F32)

            def load_wqkv_cols(eng, c0, c1):
                eng.dma_start(
                    out=wqkv_sb[:, :, c0:c1],
                    in_=wqkv[:, c0:c1].rearrange("(c p) o -> p c o", p=128))

            S = [dict() for _ in range(NB)]
            _pc = [0]

            def alloc_batch(b):
                st = S[b]
                st["xT"] = perb.tile([128, KC, N], BF16, tag="xt", name=f"xt{b}")
                st["qT"] = perb.tile([128, 4, F, SP1], BF16, tag="qT", name=f"qT{b}")
                st["kT"] = perb.tile([128, 4, F, KPAD], BF16, tag="kT", name=f"kT{b}")
                st["vfl"] = vflat_pool.tile([128, NT, INNER], BF16, tag="vfl", name=f"vfl{b}")
                st["va"] = perb.tile([128, F, INNER], BF16, tag="vfra", name=f"vfra{b}")
                st["vb"] = perb.tile([128, F, INNER], BF16, tag="vfrb", name=f"vfrb{b}")
                st["attnT"] = perb.tile([128, KC, N], BF16, tag="attnT", name=f"attnT{b}")
                oscls = clsp.tile([128, 2, 4], F32, tag="oscls", name=f"oscls{b}")
                st["oscls"] = oscls
                st["ocls"] = oscls[:, 0, :]
                st["scls"] = oscls[:, 1, :]
                st["ecc_row"] = clsp.tile([1, H], BF16, tag="eccrow", name=f"eccrow{b}")
                st["vTcls"] = clsp.tile([128, 4], F32, tag="vTcls", name=f"vTcls{b}")
                st["ecc_bc"] = clsp.tile([128, 4], F32, tag="eccbc", name=f"eccbc{b}")
                st["rcls"] = clsp.tile([128, 4], F32, tag="rcls", name=f"rcls{b}")
                st["tevc"] = clsp.tile([128, 4], F32, tag="tevc", name=f"tevc{b}")
                st["tcorr"] = clsp.tile([128, 4], F32, tag="tcorr", name=f"tcorr{b}")
                nc.vector.memset(oscls, 0.0)
                # zero the kT key-pad region so the chunk-1 S^T matmul can use
                # a full (64,128) shape (paired half-array mode). b0's runs on
                # the idle early DVE; b1's on gpsimd (gpsimd is slow but has
                # ~50us of slack before b1's first pass1).
                eng = nc.vector if b == 0 else nc.gpsimd
                eng.memset(st["kT"][:, :, :, SP1:KPAD], 0.0)

            def emit_xT_chunk(b, ci, eng):
                (t0, tl) = TCH[ci]
                eng.dma_start(
                    out=S[b]["xT"][:, :, t0:t0 + tl],
                    in_=xT[b, :, t0:t0 + tl].rearrange("(c p) t -> p c t", p=128))

            def emit_qk_item(b, oc, ci):
                isq = oc < 4
                hp = oc if isq else oc - 4
                (t0, tl) = TCH[ci]
                ps = ps_proj.tile([128, 512], F32, tag="proj")
                for kc in range(KC):
                    nc.tensor.matmul(
                        ps[:, :tl],
                        lhsT=wqkv_sb[:, kc, oc * 128:(oc + 1) * 128],
                        rhs=S[b]["xT"][:, kc, t0:t0 + tl],
                        start=(kc == 0),
                        stop=(kc == KC - 1),
                    )
                dst = S[b]["qT"] if isq else S[b]["kT"]
                eng = nc.scalar if isq else nc.vector
                cp = eng.copy if isq else eng.tensor_copy
                o0 = 0 if isq else 1
                if ci == 0:
                    cp(
                        dst[:, hp, 0:2, o0:o0 + SP],
                        ps[:, 1:tl].rearrange("p (a s) -> p a s", a=2),
                    )
                    ccol = SP if isq else 0
                    cp(
                        dst[:, hp, 0:F, ccol:ccol + 1],
                        ps[:, None, 0:1].to_broadcast([128, F, 1]),
                    )
                else:
                    cp(
                        dst[:, hp, 2 * ci:2 * ci + 2, o0:o0 + SP],
                        ps[:, :tl].rearrange("p (a s) -> p a s", a=2),
                    )

            def emit_v_item(b, t):
                m = 128 if t < NT - 1 else LAST
                vfl = S[b]["vfl"]
                ps = ps_proj.tile([128, 512], F32, tag="proj")
                for kc in range(KC):
                    nc.tensor.matmul(
                        ps[:m, :],
                        lhsT=S[b]["xT"][:, kc, 128 * t:128 * t + m],
                        rhs=wqkv_sb[:, kc, 2 * INNER:3 * INNER],
                        start=(kc == 0),
                        stop=(kc == KC - 1),
                    )
                if t % 2 == 0:
                    nc.scalar.copy(vfl[:m, t, :], ps[:m, :])
                else:
                    nc.vector.tensor_copy(vfl[:m, t, :], ps[:m, :])
                # frame-aligned rearrangement pieces sourced from this tile,
                # issue slots alternating over the gpsimd + sync queues
                for (kind, f, p0, d0, l) in rearr.get(t, []):
                    dst = S[b]["va"] if kind == "a" else S[b]["vb"]
                    _pc[0] += 1
                    eng = nc.gpsimd if _pc[0] % 2 else nc.sync
                    eng.dma_start(out=dst[d0:d0 + l, f, :], in_=vfl[p0:p0 + l, t, :])
                if t == 0:
                    nc.gpsimd.dma_start(
                        out=S[b]["va"][0:1, 0:F, :],
                        in_=vfl[0:1, 0, None, :].to_broadcast([1, F, INNER]),
                    )
                    pvt = ps_proj.tile([128, 512], F32, tag="proj")
                    for hp in range(4):
                        nc.tensor.matmul(
                            pvt[:, hp:hp + 1],
                            lhsT=vfl[0:1, 0, 128 * hp:128 * (hp + 1)],
                            rhs=ones_row[0:1, 0:1],
                            start=True, stop=True,
                        )
                    nc.vector.tensor_copy(S[b]["vTcls"], pvt[:, 0:4])

            # PSUM accumulation groups landing back-to-back on the SAME bank
            # pay a ~90ns turnaround before the second group can stream; pairs
            # on fresh banks co-run at full rate. The block emitters below
            # interleave the two steps of a block (which use different psum
            # bufs = different banks) so consecutive groups always alternate.
            def p1_block(specs):
                sts = [ps_st.tile([128, 2, 2, 256], F32, tag="st", name=f"st{i}")
                       for i, _ in enumerate(specs)]
                for chunk in (0, 1):
                    c0, c1 = (0, 128) if chunk == 0 else (128, KPAD)
                    for (g, b, hp, f), st_t in zip(specs, sts):
                        qT, kT = S[b]["qT"], S[b]["kT"]
                        for par in range(2):
                            rows = slice(64 * par, 64 * par + 64)
                            nc.tensor.matmul(
                                st_t[:, par, chunk, 0:SP1],
                                lhsT=kT[rows, hp, f, c0:c1],
                                rhs=qT[rows, hp, f, :],
                                start=True, stop=True,
                            )
                for (g, b, hp, f), st_t in zip(specs, sts):
                    ea = ea_ring[:, :, g % EARING, :, :]
                    nc.scalar.activation(ea, st_t[:, :, :, 0:SP1], EXP)
                    if f == 0:
                        for par in range(2):
                            h = 2 * hp + par
                            nc.scalar.copy(S[b]["ecc_row"][0:1, h:h + 1],
                                           ea[0:1, par, 0, SP:SP1])

            def p2_block(specs):
                pos = [ps_po.tile([128, 512], F32, tag="po", name=f"po{i}")
                       for i, _ in enumerate(specs)]
                for phase in range(4):
                    for (g, b, hp, f), po in zip(specs, pos):
                        ea = ea_ring[:, :, g % EARING, :, :]
                        va, vb = S[b]["va"], S[b]["vb"]
                        for par in range(2):
                            rows = slice(64 * par, 64 * par + 64)
                            hs = slice(DH * (2 * hp + par),
                                       DH * (2 * hp + par + 1))
                            if phase == 0:
                                nc.tensor.matmul(
                                    po[rows, 256:256 + SP1],
                                    lhsT=ones128[:, 0:64],
                                    rhs=ea[:, par, 0, :],
                                    start=True, stop=False,
                                )
                            elif phase == 1:
                                nc.tensor.matmul(
                                    po[rows, 256:256 + SP1],
                                    lhsT=ones128[0:69, 0:64],
                                    rhs=ea[0:69, par, 1, :],
                                    start=False, stop=True,
                                )
                            elif phase == 2:
                                nc.tensor.matmul(
                                    po[rows, 0:SP1],
                                    lhsT=va[:, f, hs],
                                    rhs=ea[:, par, 0, :],
                                    start=True, stop=False,
                                )
                            else:
                                nc.tensor.matmul(
                                    po[rows, 0:SP1],
                                    lhsT=vb[0:69, f, hs],
                                    rhs=ea[0:69, par, 1, :],
                                    start=False, stop=True,
                                )
                for i, ((g, b, hp, f), po) in enumerate(zip(specs, pos)):
                    rbc = small.tile([128, SP1], F32, tag="rbc",
                                     name=f"rbc{i}")
                    nc.vector.reciprocal_approx_fast(rbc, po[:, 256:256 + SP1])
                    nc.vector.tensor_tensor(
                        S[b]["attnT"][:, hp, 1 + SP * f:1 + SP * (f + 1)],
                        po[:, 0:SP], rbc[:, 0:SP], MULT,
                    )
                    nc.vector.tensor_tensor(
                        S[b]["oscls"][:, :, hp], po[:, SP:SP + 257:256],
                        S[b]["oscls"][:, :, hp], ADD,
                    )

            def cls_finalize(b):
                st = S[b]
                pec = ps_proj.tile([128, 512], F32, tag="proj")
                for hp in range(4):
                    for par in range(2):
                        h = 2 * hp + par
                        rows = slice(64 * par, 64 * par + 64)
                        nc.tensor.matmul(pec[rows, hp:hp + 1],
                                         lhsT=ones_row[0:1, 0:64],
                                         rhs=st["ecc_row"][0:1, h:h + 1],
                                         start=True, stop=True)
                nc.vector.tensor_copy(st["ecc_bc"], pec[:, 0:4])
                nc.vector.scalar_tensor_tensor(
                    st["scls"], st["ecc_bc"], -7.0, st["scls"], op0=MULT, op1=ADD,
                )
                nc.vector.reciprocal_approx_fast(st["rcls"], st["scls"])
                nc.vector.tensor_tensor(st["tevc"], st["ecc_bc"], st["vTcls"], MULT)
                nc.vector.scalar_tensor_tensor(
                    st["tcorr"], st["tevc"], -7.0, st["ocls"], op0=MULT, op1=ADD,
                )
                nc.vector.tensor_tensor(st["tcorr"], st["tcorr"], st["rcls"], MULT)
                nc.vector.tensor_copy(st["attnT"][:, 0:4, 0:1], st["tcorr"][:, :, None])

            def emit_outproj(b, t):
                m = 128 if t < NT - 1 else LAST
                ps = ps_proj.tile([128, 512], F32, tag="proj")
                for kc in range(KC):
                    nc.tensor.matmul(
                        ps[:m, :],
                        lhsT=S[b]["attnT"][:, kc, 128 * t:128 * t + m],
                        rhs=wout_sb[:, kc, :],
                        start=(kc == 0),
                        stop=(kc == KC - 1),
                    )
                osb = outp.tile([128, DIM], BF16, tag="out")
                nc.vector.tensor_tensor(osb[:m, :], ps[:m, :], bout_bc[:m, :], ADD)
                nc.sync.dma_start(out=out[b, 128 * t:128 * t + m, :], in_=osb[:m, :])

            # ---------------- stage A: b0 projection ----------
            steps_l = [(0, hp, f) for hp in range(4) for f in range(F)]
            steps_l += [(1, hp, f) for f in FORDER1 for hp in range(4)]
            alloc_batch(0)
            # input DMAs: one dma_start per logical load; first-needed loads
            # lead their queue and are split so the first v/qk matmuls can
            # start on partial data. scalar: wqkv-v, q-hp0, ci1, ci3; sync: rest.
            nc.scalar.dma_start(out=wqkv_sb[:, 0:2, 2 * INNER:3 * INNER],
                                in_=wqkv[0:256, 2 * INNER:3 * INNER].rearrange("(c p) o -> p c o", p=128))
            nc.scalar.dma_start(out=wqkv_sb[:, 2:4, 2 * INNER:3 * INNER],
                                in_=wqkv[256:512, 2 * INNER:3 * INNER].rearrange("(c p) o -> p c o", p=128))
            nc.sync.dma_start(out=S[0]["xT"][:, :, 0:128],
                              in_=xT[0, :, 0:128].rearrange("(c p) t -> p c t", p=128))
            nc.sync.dma_start(out=S[0]["xT"][:, :, 128:TCH[0][1]],
                              in_=xT[0, :, 128:TCH[0][1]].rearrange("(c p) t -> p c t", p=128))
            load_wqkv_cols(nc.scalar, 0, 128)          # q hp0
            load_wqkv_cols(nc.sync, 512, 640)          # k hp0
            emit_xT_chunk(0, 1, nc.scalar)
            emit_xT_chunk(0, 2, nc.sync)
            emit_xT_chunk(0, 3, nc.scalar)
            load_wqkv_cols(nc.sync, 128, 512)          # q hp1-3
            load_wqkv_cols(nc.sync, 640, 1024)         # k hp1-3
            nc.sync.dma_start(out=wout_sb, in_=wout[:, :].rearrange("(c p) o -> p c o", p=128))
            nc.sync.dma_start(out=bout_bc, in_=bout[:, :].to_broadcast([128, DIM]))

            # v/qk ordering follows the DMA arrival order (v-part + ci0 first)
            emit_v_item(0, 0)
            emit_v_item(0, 1)
            emit_qk_item(0, 0, 0)
            emit_qk_item(0, 4, 0)
            emit_v_item(0, 2)
            emit_v_item(0, 3)
            emit_qk_item(0, 0, 1)
            emit_qk_item(0, 4, 1)
            emit_v_item(0, 4)
            emit_v_item(0, 5)
            emit_qk_item(0, 0, 2)
            emit_qk_item(0, 4, 2)
            emit_v_item(0, 6)
            emit_v_item(0, 7)
            emit_qk_item(0, 0, 3)
            emit_qk_item(0, 4, 3)
            # hoist the first two pass1 blocks: their exps enter the ACT
            # queue ahead of the remaining stage-A copies, so the loop's
            # first pass2 blocks don't stall on a ~3us ACT backlog
            p1_block([(0,) + steps_l[0], (1,) + steps_l[1]])
            emit_v_item(0, 8)
            emit_v_item(0, 9)
            p1_block([(2,) + steps_l[2], (3,) + steps_l[3]])
            for t in range(10, NT):
                emit_v_item(0, t)

            # ---------------- main loop schedule ----------------
            sched = {}

            def add(g, fn, *a):
                sched.setdefault(g, []).append((fn, a))

            # b0 qk items hp1-3: 2 per iteration over g 0..11
            cnt = 0
            for hp in (1, 2, 3):
                for ci in range(4):
                    for oc in (hp, hp + 4):
                        add(cnt // 2, emit_qk_item, 0, oc, ci)
                        cnt += 1

            # b1: alloc + xT chunks on sync
            add(2, alloc_batch, 1)
            for ci in range(4):
                add(2 + 2 * ci, emit_xT_chunk, 1, ci, nc.sync)
            # b1 V items
            for t in range(NT):
                add(8 + t, emit_v_item, 1, t)
            # b1 qk, ci-major (frame order needs ci0 first, then ci1..3)
            cnt = 0
            for ci in range(4):
                for hp in range(4):
                    for oc in (hp, hp + 4):
                        add(14 + cnt // 2, emit_qk_item, 1, oc, ci)
                        cnt += 1

            # b0 outproj: hp3 step for frame f is at g = 24+f, pass2 at 26+f;
            # +1 extra bucket of slack so the PE doesn't catch the DVE attnT
            # writes mid-flight
            def fmax0(t):
                return min(F - 1, (128 * (t + 1) - 2) // SP)
            add(34, cls_finalize, 0)
            for t in range(1, NT):
                add(29 + fmax0(t), emit_outproj, 0, t)
            add(37, emit_outproj, 0, 0)

            # b1 outproj: last hp step of frame at position k is g = G1+4k+3
            fpos1 = {f: k for k, f in enumerate(FORDER1)}
            for t in range(2, NT):
                # frames touched by tile t: fmin..fmax
                fmin = max(0, (128 * t - 1) // SP)
                fmx = fmax0(t)
                kmax = max(fpos1[f] for f in range(fmin, fmx + 1))
                add(min(62, G1 + 4 * kmax + 3 + LAG + 3), emit_outproj, 1, t)
            add(NSTEP_L + LAG, cls_finalize, 1)
            add(NSTEP_L + LAG, emit_outproj, 1, 1)
            add(NSTEP_L + LAG + 1, emit_outproj, 1, 0)

            # block-2 iteration: two attention steps run back-to-back (24
            # contiguous PE matmuls) before the projection items interleave;
            # mixing proj items between single steps measured ~35ns/mm slower.
            # EARING=4 exactly covers the 4 live exp slots this creates.
            for gb in range(0, NSTEP_L + LAG + 2, 2):
                items0 = sched.pop(gb, [])
                items1 = sched.pop(gb + 1, [])
                if items0:
                    fn, a = items0.pop(0)
                    fn(*a)
                p1specs = [(g,) + steps_l[g] for g in (gb, gb + 1)
                           if 4 <= g < NSTEP_L]
                if p1specs:
                    p1_block(p1specs)
                p2specs = [(g - LAG,) + steps_l[g - LAG] for g in (gb, gb + 1)
                           if g >= LAG and g - LAG < NSTEP_L]
                if p2specs:
                    p2_block(p2specs)
                for (fn, a) in items0 + items1:
                    fn(*a)
            for g in sorted(sched):
                for (fn, a) in sched[g]:
                    fn(*a)

    nc.finalize()
    return nc


_CACHE = {}


def _get_nc():
    if "nc" not in _CACHE:
        _CACHE["nc"] = build_nc()
    return _CACHE["nc"]


def prepare_in_maps(x, f, W_qkv, W_out, b_out):
    assert int(f) == F
    x = np.asarray(x, dtype=np.float32)
    W_qkv = np.asarray(W_qkv, dtype=np.float32).copy()
    W_out = np.asarray(W_out, dtype=np.float32)
    b_out = np.asarray(b_out, dtype=np.float32)
    W_qkv[:, :INNER] *= DH ** -0.5
    wqkv_bf = W_qkv.astype(NPBF)
    wout_bf = W_out.astype(NPBF)
    bout_np = b_out.reshape(1, DIM)
    xT = np.ascontiguousarray(x.transpose(0, 2, 1)).astype(NPBF)
    in_maps = []
    for c in range(NCORES):
        in_maps.append({
            "xT": np.ascontiguousarray(xT[NB * c:NB * (c + 1)]),
            "wqkv": wqkv_bf,
            "wout": wout_bf,
            "bout": bout_np,
        })
    return in_maps


def kernel(x, f, W_qkv, W_out, b_out):
    nc = _get_nc()
    in_maps = prepare_in_maps(x, f, W_qkv, W_out, b_out)
    res = bass_utils.run_bass_kernel_spmd(nc, in_maps, list(range(NCORES)))
    return np.concatenate(
        [np.asarray(r["out"], dtype=np.float32) for r in res.results], axis=0)
